# revision 18
# baseline (speedup 1.0000x reference)
"""Trainium2 Bass kernel for nn_BoundleAdjustment (2M observations).

Two launches on all 8 NeuronCores (observations data-parallel, M/8 per core):

Launch A (device): converts the 4096-row pose quaternion table into
rotation matrices with 12 wide DVE instructions (host pre-replicates the
quat component pairs so all 10 products are one tensor_tensor).  One DMA
in, one DMA out.

Host staging (indexing only): gathers the derived R table rows, pose
translations and patch rows by poses_idx/patch_idx and lays the
per-observation records out as fp16 planes, chunk-major so each chunk is
one contiguous DMA per partition.

Launch B (device): NCH chunks; per chunk one input DMA ([128, 19*cc] fp16)
and one output DMA ([128, 3*cc] fp16).  Rotation r = R*p + t is one
broadcast multiply (9cc wide, fp16 2x) plus three batched adds; squares /
sqrts / sign / arctans on the Scalar engine grouped in two phases per
chunk-group so the activation function table (sqrt-set vs arctan-set)
switches only twice per group; rho2/r2 and part of the residual run on the
Pool (gpsimd) engine; az/el division via two vector-engine reciprocals.
"""

import numpy as np

M = 2097152
NCORES = 8
N = M // NCORES          # 262144 obs per core
P = 128
COLS = N // P            # 2048
# variable chunk sizes: small ramp-in (compute starts early), small tail
CHUNKS = [128, 128, 256, 384, 512, 512, 128]
assert sum(CHUNKS) == COLS
NGROUPS = 2              # act-table phase groups (sqrt-set then arctan-set)
GBOUND = [0, 5, len(CHUNKS)]   # group g = chunks GBOUND[g]:GBOUND[g+1]
NPOSE = 4096
PC = NPOSE // P          # 32 pose cols per partition
NPL = 19                 # planes per obs in launch B
PI = float(np.pi)

# launch A: product operand replication (quaternion components 0..3 = x,y,z,w)
_QA = [0, 1, 2, 3, 0, 0, 1, 3, 3, 3]   # in0 components
_QB = [0, 1, 2, 3, 1, 2, 2, 2, 1, 0]   # in1 components
# prods -> [xx yy zz ww | xy xz yz | wz wy wx]
# A output plane order: [R00,R11,R22, R01,R20,R12, R10,R02,R21]
# map to row-major R order R00,R01,R02,R10,R11,R12,R20,R21,R22:
_RPERM = [0, 3, 7, 6, 1, 5, 4, 8, 2]

_CACHE = {}


def _build_posetab():
    import concourse.tile as tile
    from concourse import bacc, mybir

    nc = bacc.Bacc("TRN2", target_bir_lowering=False, debug=False,
                   num_devices=NCORES)
    f32, f16 = mybir.dt.float32, mybir.dt.float16
    OP = mybir.AluOpType
    q_d = nc.declare_dram_parameter("q", [P, 2 * 10 * PC], f32, isOutput=False)
    r_d = nc.declare_dram_parameter("rtab", [P, 9 * PC], f16, isOutput=True)

    with tile.TileContext(nc) as tc:
        with tc.tile_pool(name="pp", bufs=1) as pp, \
             nc.allow_low_precision(reason="fp16 R table staging"):
            vec = nc.vector

            def T(cols, dt=f32, nm=""):
                return pp.tile([P, cols], dt, tag=nm, name=nm)

            ab = T(2 * 10 * PC, nm="ab")
            nc.sync.dma_start(ab[:], q_d[:, :])
            prods = T(10 * PC, nm="prods")
            vec.tensor_tensor(out=prods[:], in0=ab[:, 0:10 * PC],
                              in1=ab[:, 10 * PC:20 * PC], op=OP.mult)
            s2 = T(2 * PC, nm="s2")
            vec.tensor_tensor(out=s2[:], in0=prods[:, 0:2 * PC],
                              in1=prods[:, 2 * PC:4 * PC], op=OP.add)
            s = T(PC, nm="s")
            vec.tensor_tensor(out=s[:], in0=s2[:, 0:PC], in1=s2[:, PC:2 * PC],
                              op=OP.add)
            h = T(PC, nm="h")
            vec.tensor_scalar(out=h[:], in0=s[:], scalar1=0.5, scalar2=None,
                              op0=OP.mult)
            u = T(PC, nm="u")
            vec.reciprocal(u[:], h[:])       # u = 2/|q|^2
            ub = u[:].unsqueeze(1).broadcast_to([P, 3, PC])

            out = T(9 * PC, f16, nm="o")
            ds = T(3 * PC, nm="ds")
            wwb = prods[:, 3 * PC:4 * PC].unsqueeze(1).broadcast_to([P, 3, PC])
            vec.tensor_tensor(out=ds[:].rearrange("p (a c) -> p a c", a=3),
                              in0=prods[:, 0:3 * PC].rearrange(
                                  "p (a c) -> p a c", a=3),
                              in1=wwb, op=OP.add)
            du = T(3 * PC, nm="du")
            vec.tensor_tensor(out=du[:].rearrange("p (a c) -> p a c", a=3),
                              in0=ds[:].rearrange("p (a c) -> p a c", a=3),
                              in1=ub, op=OP.mult)
            # diag R = u*(own^2+ww) - 1
            vec.tensor_scalar(out=out[:, 0:3 * PC], in0=du[:], scalar1=-1.0,
                              scalar2=None, op0=OP.add)
            moff = T(3 * PC, nm="moff")
            vec.tensor_tensor(out=moff[:], in0=prods[:, 4 * PC:7 * PC],
                              in1=prods[:, 7 * PC:10 * PC], op=OP.subtract)
            mplus = T(3 * PC, nm="mplus")
            vec.tensor_tensor(out=mplus[:], in0=prods[:, 4 * PC:7 * PC],
                              in1=prods[:, 7 * PC:10 * PC], op=OP.add)
            vec.tensor_tensor(out=out[:, 3 * PC:6 * PC].rearrange(
                                  "p (a c) -> p a c", a=3),
                              in0=moff[:].rearrange("p (a c) -> p a c", a=3),
                              in1=ub, op=OP.mult)
            vec.tensor_tensor(out=out[:, 6 * PC:9 * PC].rearrange(
                                  "p (a c) -> p a c", a=3),
                              in0=mplus[:].rearrange("p (a c) -> p a c", a=3),
                              in1=ub, op=OP.mult)
            nc.sync.dma_start(r_d[:, :], out[:])
    nc.finalize()
    return nc


def _build_main():
    import concourse.tile as tile
    from concourse import bacc, mybir

    nc = bacc.Bacc("TRN2", target_bir_lowering=False, debug=False,
                   num_devices=NCORES)
    f32, f16 = mybir.dt.float32, mybir.dt.float16
    OP = mybir.AluOpType
    AF = mybir.ActivationFunctionType
    in_d = nc.declare_dram_parameter("in", [P, NPL * COLS], f16,
                                     isOutput=False)
    out_d = nc.declare_dram_parameter("out", [P, 3 * COLS], f16,
                                      isOutput=True)
    NCH = len(CHUNKS)

    with tile.TileContext(nc) as tc:
        with tc.tile_pool(name="pp", bufs=1) as pp, \
             nc.allow_low_precision(reason="fp16 stream, fp32-safe sqrt path"):
            vec, act, pool = nc.vector, nc.scalar, nc.gpsimd

            insA, insB, aes, corrs, outs, rngs = [], [], [], [], [], []
            state = {}
            off = 0
            for c, cc in enumerate(CHUNKS):
                # split: A = R/t/p planes (dead after rotation), B = X/Y/Z/w
                ta = pp.tile([P, 15 * cc], f16, tag="inA", bufs=3,
                             name=f"inA{c}")
                nc.sync.dma_start(ta[:], in_d[:, NPL * off:NPL * off + 15 * cc])
                tb = pp.tile([P, 4 * cc], f16, tag=f"inB{c}", name=f"inB{c}")
                nc.sync.dma_start(tb[:], in_d[:, NPL * off + 15 * cc:
                                               NPL * (off + cc)])
                insA.append(ta)
                insB.append(tb)
                aes.append(pp.tile([P, 2 * cc], f16, tag=f"ae{c}",
                                   name=f"ae{c}"))
                corrs.append(pp.tile([P, cc], f16, tag=f"co{c}",
                                     name=f"co{c}"))
                outs.append(pp.tile([P, 3 * cc], f16, tag=f"o{c}",
                                    name=f"o{c}"))
                rngs.append(None)
                off += cc

            def phase1(c, fast_tail=False):
                cc = CHUNKS[c]
                IN = insA[c]
                INB = insB[c]
                res = vec if fast_tail else pool
                R = IN[:, 0:9 * cc].rearrange("p (a x) -> p a x", a=3)
                tv = IN[:, 9 * cc:12 * cc].rearrange("p (a x) -> p a x", a=3)
                pv = IN[:, 12 * cc:15 * cc].unsqueeze(1).broadcast_to(
                    [P, 3, 3 * cc])
                prods = pp.tile([P, 9 * cc], f16, tag="prods", bufs=3,
                                name=f"pr{c}")
                vec.tensor_tensor(out=prods[:].rearrange("p (a x) -> p a x",
                                                         a=3),
                                  in0=R, in1=pv, op=OP.mult)
                prv = prods[:].rearrange("p (a k x) -> p a k x", a=3, k=3)
                s1 = pp.tile([P, 3 * cc], f16, tag="s1", bufs=3, name=f"s1_{c}")
                s1v = s1[:].rearrange("p (a x) -> p a x", a=3)
                vec.tensor_tensor(out=s1v, in0=prv[:, :, 0, :],
                                  in1=prv[:, :, 1, :], op=OP.add)
                s2 = pp.tile([P, 3 * cc], f16, tag="s2", bufs=3, name=f"s2_{c}")
                s2v = s2[:].rearrange("p (a x) -> p a x", a=3)
                vec.tensor_tensor(out=s2v, in0=prv[:, :, 2, :], in1=tv,
                                  op=OP.add)
                # rvec layout: [ry | rz | rx | rho | rng] (sqrt fills 3-4)
                rvec = pp.tile([P, 5 * cc], f16, tag="rv", bufs=3,
                               name=f"rv{c}")
                vec.tensor_tensor(out=rvec[:, 0:3 * cc], in0=s1[:], in1=s2[:],
                                  op=OP.add)
                ry = rvec[:, 0:cc]
                rx = rvec[:, 2 * cc:3 * cc]

                sq = pp.tile([P, 3 * cc], f16, tag="sq", bufs=3, name=f"sq{c}")
                act.activation(sq[:], rvec[:, 0:3 * cc], AF.Square)
                rr = pp.tile([P, 2 * cc], f16, tag="rr", bufs=3,
                             name=f"rr{c}")
                res.tensor_tensor(out=rr[:, 0:cc], in0=sq[:, 2 * cc:3 * cc],
                                  in1=sq[:, 0:cc], op=OP.add)
                res.tensor_tensor(out=rr[:, cc:2 * cc], in0=rr[:, 0:cc],
                                  in1=sq[:, cc:2 * cc], op=OP.add)
                # rho first (unblocks the reciprocal), then rng
                act.activation(rvec[:, 3 * cc:4 * cc], rr[:, 0:cc], AF.Sqrt)
                act.activation(rvec[:, 4 * cc:5 * cc], rr[:, cc:2 * cc],
                               AF.Sqrt)
                rng = rvec[:, 4 * cc:5 * cc]
                rngs[c] = rvec

                state[c] = (rvec, rng)

            def phase1b(c, fast_tail=False):
                cc = CHUNKS[c]
                INB = insB[c]
                res = vec if fast_tail else pool
                rvec, rng = state[c]
                ry = rvec[:, 0:cc]
                rx = rvec[:, 2 * cc:3 * cc]
                sgn = pp.tile([P, cc], f16, tag="sgn", bufs=3, name=f"sgn{c}")
                act.activation(sgn[:], ry, AF.Sign)
                iab = pp.tile([P, 2 * cc], f16, tag="iab", bufs=3,
                              name=f"iab{c}")
                vec.reciprocal(iab[:], rvec[:, 2 * cc:4 * cc])
                # a1 = ry/rx, e1 = rz/rho
                vec.tensor_tensor(out=aes[c][:], in0=rvec[:, 0:2 * cc],
                                  in1=iab[:], op=OP.mult)
                c0 = pp.tile([P, cc], f16, tag="c0", bufs=3, name=f"c0_{c}")
                vec.tensor_scalar(out=c0[:], in0=rx, scalar1=0.0, scalar2=PI,
                                  op0=OP.is_lt, op1=OP.mult)
                corr = pp.tile([P, cc], f16, tag="corr", bufs=3,
                               name=f"corr{c}")
                res.tensor_tensor(out=corr[:], in0=c0[:], in1=sgn[:],
                                  op=OP.mult)
                # yc = Y - corr  (phase 2 then needs only az0 - yc)
                res.tensor_tensor(out=corrs[c][:], in0=INB[:, cc:2 * cc],
                                  in1=corr[:], op=OP.subtract)
                # x-residual now (rng ready): ox = (rng - X) * w
                dx = pp.tile([P, cc], f16, tag="dx", bufs=3, name=f"dx{c}")
                vec.tensor_tensor(out=dx[:], in0=rng,
                                  in1=INB[:, 0:cc], op=OP.subtract)
                res.tensor_tensor(out=outs[c][:, 0:cc], in0=dx[:],
                                  in1=INB[:, 3 * cc:4 * cc], op=OP.mult)

            def phase2(c, gate, fast_tail=False):
                cc = CHUNKS[c]
                INB = insB[c]
                res = vec if fast_tail else pool
                azel = pp.tile([P, 2 * cc], f16, tag="azel", bufs=3,
                               name=f"azel{c}")
                # bias is a zero-valued [P,1] gate tile: forces this arctan
                # after the group's last phase-1 sqrt so the act-func table
                # switches only once per phase (semantically a no-op).
                act.activation(azel[:], aes[c][:], AF.Arctan,
                               bias=gate[:, 0:1])
                dyz = pp.tile([P, 2 * cc], f16, tag="dyz", bufs=3,
                              name=f"dyz{c}")
                # dy = az0 - (Y - corr);  dz = el - Z
                vec.tensor_tensor(out=dyz[:, 0:cc], in0=azel[:, 0:cc],
                                  in1=corrs[c][:], op=OP.subtract)
                vec.tensor_tensor(out=dyz[:, cc:2 * cc], in0=azel[:, cc:2 * cc],
                                  in1=INB[:, 2 * cc:3 * cc], op=OP.subtract)
                wb = INB[:, 3 * cc:4 * cc].unsqueeze(1).broadcast_to(
                    [P, 2, cc])
                res.tensor_tensor(out=outs[c][:, cc:3 * cc].rearrange(
                                      "p (a x) -> p a x", a=2),
                                  in0=dyz[:].rearrange("p (a x) -> p a x",
                                                       a=2),
                                  in1=wb, op=OP.mult)
                off3 = 3 * sum(CHUNKS[:c])
                nc.sync.dma_start(out_d[:, off3:off3 + 3 * CHUNKS[c]],
                                  outs[c][:])

            for g in range(NGROUPS):
                lo, hi = GBOUND[g], GBOUND[g + 1]
                last = g == NGROUPS - 1
                for c in range(lo, hi):
                    phase1(c, fast_tail=last)
                    phase1b(c, fast_tail=last)
                gate = pp.tile([P, 1], f16, tag=f"gate{g}", name=f"gate{g}")
                cl = CHUNKS[hi - 1]
                vec.tensor_scalar(out=gate[:],
                                  in0=rngs[hi - 1][:, 4 * cl:4 * cl + 1],
                                  scalar1=0.0, scalar2=None, op0=OP.mult)
                for c in range(lo, hi):
                    phase2(c, gate, fast_tail=last)
    nc.finalize()
    return nc


def _get(name, builder):
    if name not in _CACHE:
        _CACHE[name] = builder()
    return _CACHE[name]


def stage_A(poses):
    """[P, 2*10*PC] f32: replicated quat component products for launch A."""
    q = poses[:, 3:7].reshape(P, PC, 4)          # [p, c, comp]
    qa = np.stack([q[:, :, k] for k in _QA], axis=1)   # [P, 10, PC]
    qb = np.stack([q[:, :, k] for k in _QB], axis=1)
    return np.ascontiguousarray(
        np.concatenate([qa, qb], axis=1).reshape(P, 2 * 10 * PC)
    ).astype(np.float32)


def unpack_A(rtab_raw):
    """launch A output [P, 9*PC] f16 -> R table [NPOSE, 9] row-major f32."""
    r = np.asarray(rtab_raw).astype(np.float32).reshape(P, 9, PC)
    r = r.transpose(0, 2, 1).reshape(NPOSE, 9)   # pose = p*PC + c
    return r[:, _RPERM]


AXORD = [1, 2, 0]        # axis processing order (y, z, x)


def stage_B(r9, t3, pts, target, w):
    """Per-obs 19-plane fp16 staging, chunk-major: [NCORES, P, NPL*COLS]."""
    rperm = [3 * a + k for a in AXORD for k in range(3)]
    D = np.concatenate([r9[:, rperm], t3[:, AXORD], pts, target, w],
                       axis=1)                             # [M, 19]
    D = D.astype(np.float16).reshape(NCORES, P, COLS, NPL)
    blocks = []
    off = 0
    for cc in CHUNKS:
        blk = D[:, :, off:off + cc, :].transpose(0, 1, 3, 2)  # [.., NPL, cc]
        blocks.append(blk.reshape(NCORES, P, NPL * cc))
        off += cc
    return np.ascontiguousarray(np.concatenate(blocks, axis=2))


def unpack_B(res_list):
    out = np.stack([np.asarray(res_list[c]["out"]) for c in range(NCORES)])
    cols = []
    off = 0
    for cc in CHUNKS:
        blk = out[:, :, 3 * off:3 * (off + cc)].reshape(NCORES, P, 3, cc)
        cols.append(blk.transpose(0, 1, 3, 2))       # [cores, P, cc, 3]
        off += cc
    full = np.concatenate(cols, axis=2)              # [cores, P, COLS, 3]
    return np.ascontiguousarray(full.reshape(M, 3)).astype(np.float32)


def kernel(poses, patch_coords, elevation_angle, poses_idx, patch_idx,
           target_coords, weights):
    from concourse.bass_utils import run_bass_kernel_spmd

    poses = np.asarray(poses, dtype=np.float32)
    patch_coords = np.asarray(patch_coords, dtype=np.float32)
    elevation_angle = np.asarray(elevation_angle, dtype=np.float32)
    target_coords = np.asarray(target_coords, dtype=np.float32)
    weights = np.asarray(weights, dtype=np.float32)
    pid = np.asarray(poses_idx).astype(np.int64)
    qid = np.asarray(patch_idx).astype(np.int64)

    # ---- launch A: pose quaternions -> rotation matrices (device) ----
    qAB = stage_A(poses)
    ncA = _get("A", _build_posetab)
    resA = run_bass_kernel_spmd(ncA, [{"q": qAB} for _ in range(NCORES)],
                                list(range(NCORES)))
    rtab = unpack_A(resA.results[0]["rtab"])            # [4096, 9]

    # ---- host: staging (indexing only) ----
    big = stage_B(rtab[pid], poses[pid, 0:3],
                  np.concatenate([patch_coords[qid], elevation_angle[qid]],
                                 axis=1),
                  target_coords, weights)

    # ---- launch B: streaming rotate + polar + residual ----
    ncB = _get("B", _build_main)
    resB = run_bass_kernel_spmd(ncB, [{"in": big[c]} for c in range(NCORES)],
                                list(range(NCORES)))
    return unpack_B(resB.results)
